# revision 4
# baseline (speedup 1.0000x reference)
"""Causal single-head attention (B=4, S=4096, D=768, fp32) on 8 Trainium2 NeuronCores.

Sharding: 2 cores per batch. The 16 query blocks (256 rows each) of a batch are
split between the pair so causal work balances (snake pairing); every core runs
the SAME compiled program — all per-core differences (which query rows, causal
masks) are shipped as data.

On-device algorithm (flash-attention style, fp32r matmuls = full PE speed with
~tf32 accuracy):
  phase 1: xT via PE transpose; project K^T and V(+ones col) to DRAM scratch,
           Q^T (own queries only) resident in SBUF.
  phase 2: stream K^T/V back in blocks of 4 k-chunks (kc-block OUTER loop, so
           each k block is read once); for each query slot still active, compute
           S^T = K Q^T in PSUM, exp on ScalarE (scale folded), mask via shipped
           per-core mask tiles on the final block, accumulate U = W @ [V|1] in
           PSUM, flush-add into an SBUF accumulator.
  phase 3: out = U[:, :768] * (1 / U[:, 768]) per row, DMA out.
"""

import numpy as np

import concourse.bacc as bacc
import concourse.mybir as mybir
import concourse.tile as tile
from concourse.bass_utils import run_bass_kernel_spmd
from concourse.masks import make_identity

B, S, D = 4, 4096, 768
P = 128
DC = D // P            # 6 feature chunks
NKC = S // P           # 32 key chunks
QB = 256               # query block width
NSLOT = 8              # query blocks per core (8 * 256 = 2048 queries)
SQ = NSLOT * QB        # 2048
KB = 4                 # key chunks per streamed block
NKB = NKC // KB        # 8
VW = 772               # V row width: 768 + ones col + pad
SM_SCALE = float(1.0 / np.sqrt(D))
N_CORES = 8

# Snake pairing of the 16 global query blocks between the two cores of a batch:
# slot j holds global block 2j or 2j+1; both cores run ceil-padded trip counts.
ASSIGN = [
    [(2 * j + 1) if j % 2 == 0 else (2 * j) for j in range(NSLOT)],  # role 0
    [(2 * j) if j % 2 == 0 else (2 * j + 1) for j in range(NSLOT)],  # role 1
]

ACT_F32R = True  # ScalarE may write float32r (rounding) — flip if compile rejects

_CACHE = {}


def _round_copy(nc, dst, src):
    if ACT_F32R:
        nc.scalar.copy(dst, src)
    else:
        nc.vector.tensor_copy(dst, src)


def _build_module():
    dt = mybir.dt
    f32, f32r = dt.float32, dt.float32r
    nc = bacc.Bacc("TRN2", target_bir_lowering=False, debug=False,
                   num_devices=N_CORES)

    x_d = nc.dram_tensor("x", [S, D], f32, kind="ExternalInput").ap()
    xq_d = nc.dram_tensor("xq", [SQ, D], f32, kind="ExternalInput").ap()
    wq_d = nc.dram_tensor("Wq", [D, D], f32, kind="ExternalInput").ap()
    wk_d = nc.dram_tensor("Wk", [D, D], f32, kind="ExternalInput").ap()
    wv_d = nc.dram_tensor("Wv", [D, D], f32, kind="ExternalInput").ap()
    mask_d = nc.dram_tensor("masks", [NSLOT, KB, P, QB], f32r,
                            kind="ExternalInput").ap()
    out_d = nc.dram_tensor("out", [SQ, D], f32, kind="ExternalOutput").ap()

    x_r = x_d.rearrange("(a p) d -> p a d", p=P)      # [128, 32, 768]
    xq_r = xq_d.rearrange("(a p) d -> p a d", p=P)    # [128, 16, 768]
    out_r = out_d.rearrange("(a p) d -> p a d", p=P)  # [128, 16, 768]

    Exp = mybir.ActivationFunctionType.Exp

    with tile.TileContext(nc) as tc:
        with tc.tile_pool(name="singles", bufs=1) as singles, \
             tc.tile_pool(name="dram", bufs=1, space="DRAM") as dram:
            ident = singles.tile([P, P], f32)
            make_identity(nc, ident)
            qt = singles.tile([P, DC, SQ], f32r)      # Q^T resident
            kt_t = dram.tile([P, DC, S], f32r)        # K^T scratch
            v_t = dram.tile([P, NKC, VW], f32r)       # V(+ones) scratch

            # ---------------- phase 1: projections ----------------
            with tc.tile_pool(name="wload", bufs=1) as wload, \
                 tc.tile_pool(name="weights", bufs=1) as weights, \
                 tc.tile_pool(name="p1", bufs=2) as p1, \
                 tc.tile_pool(name="stage", bufs=3) as stage, \
                 tc.tile_pool(name="pst", bufs=2, space="PSUM") as pst, \
                 tc.tile_pool(name="psp", bufs=2, space="PSUM") as psp, \
                 tc.tile_pool(name="psv", bufs=2, space="PSUM") as psv:
                w_r = {}
                for name, wd in (("wq", wq_d), ("wk", wk_d), ("wv", wv_d)):
                    wf = wload.tile([P, DC, D], f32, tag="wf")
                    nc.sync.dma_start(out=wf, in_=wd.rearrange("(c p) e -> p c e", p=P))
                    wr = weights.tile([P, DC, D], f32r, tag="w_" + name)
                    nc.vector.tensor_copy(wr, wf)
                    w_r[name] = wr

                def proj_block(src_r, a0, is_q):
                    # one 512-row block of x (or xq): transpose + project
                    xs = p1.tile([P, 4, D], f32, tag="xs")
                    nc.sync.dma_start(out=xs, in_=src_r[:, a0:a0 + 4, :])
                    xt = p1.tile([P, DC, 4 * P], f32r, tag="xt")
                    for sc in range(4):
                        for c in range(DC):
                            pt = pst.tile([P, P], f32, tag="pt")
                            nc.tensor.transpose(pt, xs[:, sc, c * P:(c + 1) * P], ident)
                            nc.vector.tensor_copy(xt[:, c, sc * P:(sc + 1) * P], pt)
                    if is_q:
                        w = w_r["wq"]
                        for ec in range(DC):
                            pp = psp.tile([P, 4 * P], f32, tag="pp")
                            for c in range(DC):
                                nc.tensor.matmul(pp, w[:, c, ec * P:(ec + 1) * P],
                                                 xt[:, c, :],
                                                 start=(c == 0), stop=(c == DC - 1))
                            _round_copy(nc, qt[:, ec, a0 * P:a0 * P + 4 * P], pp)
                    else:
                        w = w_r["wk"]
                        for ec in range(DC):
                            pp = psp.tile([P, 4 * P], f32, tag="pp")
                            for c in range(DC):
                                nc.tensor.matmul(pp, w[:, c, ec * P:(ec + 1) * P],
                                                 xt[:, c, :],
                                                 start=(c == 0), stop=(c == DC - 1))
                            stk = stage.tile([P, 4 * P], f32r, tag="stk")
                            _round_copy(nc, stk, pp)
                            nc.sync.dma_start(out=kt_t[:, ec, a0 * P:a0 * P + 4 * P],
                                              in_=stk)
                        wv = w_r["wv"]
                        for sc in range(4):
                            pv = psv.tile([P, VW], f32, tag="pv")
                            for c in range(DC):
                                lhs = xt[:, c, sc * P:(sc + 1) * P]
                                nc.tensor.matmul(pv[:, 0:512], lhs, wv[:, c, 0:512],
                                                 start=(c == 0), stop=(c == DC - 1))
                                nc.tensor.matmul(pv[:, 512:768], lhs, wv[:, c, 512:768],
                                                 start=(c == 0), stop=(c == DC - 1))
                            sv = stage.tile([P, VW], f32r, tag="sv")
                            _round_copy(nc, sv[:, 0:768], pv[:, 0:768])
                            nc.vector.memset(sv[:, 768:769].bitcast(mybir.dt.float32), 1.0)
                            nc.vector.memset(sv[:, 769:VW].bitcast(mybir.dt.float32), 0.0)
                            nc.sync.dma_start(out=v_t[:, a0 + sc, :], in_=sv)

                for blk in range(SQ // (4 * P)):
                    proj_block(xq_r, blk * 4, True)
                for blk in range(S // (4 * P)):
                    proj_block(x_r, blk * 4, False)

            # ---------------- phase 2: attention ----------------
            with tc.tile_pool(name="uaccp", bufs=1) as uaccp, \
                 tc.tile_pool(name="ring", bufs=2) as ring, \
                 tc.tile_pool(name="wtp", bufs=3) as wtp, \
                 tc.tile_pool(name="mring", bufs=2) as mring, \
                 tc.tile_pool(name="fin", bufs=2) as fin, \
                 tc.tile_pool(name="pss", bufs=3, space="PSUM") as pss, \
                 tc.tile_pool(name="psu", bufs=1, space="PSUM") as psu:
                uacc = uaccp.tile([P, 2 * NSLOT, VW], f32)
                for kb in range(NKB):
                    ktb = ring.tile([P, DC, KB * P], f32r, tag="ktb")
                    nc.sync.dma_start(out=ktb,
                                      in_=kt_t[:, :, kb * KB * P:(kb + 1) * KB * P])
                    vb = ring.tile([P, KB, VW], f32r, tag="vb")
                    nc.sync.dma_start(out=vb, in_=v_t[:, kb * KB:(kb + 1) * KB, :])
                    for j in range(kb, NSLOT):
                        mb = None
                        if kb == j:
                            mb = mring.tile([P, KB, QB], f32r, tag="mb")
                            nc.sync.dma_start(
                                out=mb, in_=mask_d[j].rearrange("t p f -> p t f"))
                        pu0 = psu.tile([P, VW], f32, tag="pu0")
                        pu1 = psu.tile([P, VW], f32, tag="pu1")
                        pus = (pu0, pu1)
                        for t in range(KB):
                            ps = pss.tile([P, QB], f32, tag="ps")
                            for c in range(DC):
                                nc.tensor.matmul(ps, ktb[:, c, t * P:(t + 1) * P],
                                                 qt[:, c, j * QB:(j + 1) * QB],
                                                 start=(c == 0), stop=(c == DC - 1))
                            wt = wtp.tile([P, QB], f32r, tag="wt")
                            if ACT_F32R:
                                nc.scalar.activation(wt, ps, Exp, scale=SM_SCALE)
                            else:
                                wtf = wtp.tile([P, QB], f32, tag="wtf")
                                nc.scalar.activation(wtf, ps, Exp, scale=SM_SCALE)
                                nc.vector.tensor_copy(wt, wtf)
                            if mb is not None:
                                nc.vector.tensor_mul(wt, wt, mb[:, t, :])
                            for qc in range(2):
                                lhs = wt[:, qc * P:(qc + 1) * P]
                                nc.tensor.matmul(pus[qc][:, 0:512], lhs,
                                                 vb[:, t, 0:512],
                                                 start=(t == 0), stop=(t == KB - 1))
                                nc.tensor.matmul(pus[qc][:, 512:VW], lhs,
                                                 vb[:, t, 512:VW],
                                                 start=(t == 0), stop=(t == KB - 1))
                        for qc in range(2):
                            dst = uacc[:, 2 * j + qc, 0:769]
                            if kb == 0:
                                nc.scalar.copy(dst, pus[qc][:, 0:769])
                            else:
                                nc.vector.tensor_add(dst, dst, pus[qc][:, 0:769])

                # ---------------- phase 3: normalize + store ----------------
                for j in range(NSLOT):
                    for qc in range(2):
                        sl = 2 * j + qc
                        zr = fin.tile([P, 1], f32, tag="zr")
                        nc.vector.reciprocal(zr, uacc[:, sl, 768:769])
                        ob = fin.tile([P, D], f32, tag="ob")
                        nc.scalar.mul(ob, uacc[:, sl, 0:768], zr)
                        nc.sync.dma_start(out=out_r[:, sl, :], in_=ob)

    nc.compile()
    return nc


def _get_module():
    if "nc" not in _CACHE:
        _CACHE["nc"] = _build_module()
    return _CACHE["nc"]


def _build_masks(chunks):
    m = np.zeros((NSLOT, KB, P, QB), np.float32)
    prow = np.arange(P)[:, None]
    fcol = np.arange(QB)[None, :]
    for j, g in enumerate(chunks):
        for t in range(KB):
            kc = KB * j + t
            m[j, t] = (prow <= fcol + (g * QB - kc * P)).astype(np.float32)
    return m


def _make_in_maps(inputs):
    x = np.asarray(inputs["x"], np.float32)
    Wq = np.ascontiguousarray(np.asarray(inputs["Wq"], np.float32))
    Wk = np.ascontiguousarray(np.asarray(inputs["Wk"], np.float32))
    Wv = np.ascontiguousarray(np.asarray(inputs["Wv"], np.float32))
    in_maps = []
    for c in range(N_CORES):
        b, r = c // 2, c % 2
        chunks = ASSIGN[r]
        xb = np.ascontiguousarray(x[b])
        xq = np.ascontiguousarray(
            xb.reshape(S // QB, QB, D)[chunks].reshape(SQ, D))
        in_maps.append({
            "x": xb, "xq": xq, "Wq": Wq, "Wk": Wk, "Wv": Wv,
            "masks": _build_masks(chunks),
        })
    return in_maps


def _run(inputs, trace=False, trace_kwargs=None):
    nc = _get_module()
    in_maps = _make_in_maps(inputs)

    kw = {}
    if trace:
        kw["trace"] = True
        kw["trace_cores"] = list(range(N_CORES))
        kw["stitch_traces"] = True
        if trace_kwargs:
            kw["trace_kwargs"] = trace_kwargs
    res = run_bass_kernel_spmd(nc, in_maps, core_ids=list(range(N_CORES)), **kw)

    out = np.empty((B, S, D), np.float32)
    for c in range(N_CORES):
        b, r = c // 2, c % 2
        o = res.results[c]["out"].reshape(NSLOT, QB, D)
        for j, g in enumerate(ASSIGN[r]):
            out[b, g * QB:(g + 1) * QB] = o[j]
    return out, res


def kernel(**inputs) -> np.ndarray:
    out, _ = _run(inputs, trace=False)
    return out


# revision 5
# speedup vs baseline: 17.0779x; 17.0779x over previous
"""Causal single-head attention (B=4, S=4096, D=768, fp32) on 8 Trainium2 NeuronCores.

Sharding: 2 cores per batch. The 16 query blocks (256 rows each) of a batch are
split between the pair so causal work balances (snake pairing); every core runs
the SAME compiled program — all per-core differences (which query rows, causal
masks) are shipped as data.

On-device algorithm (flash-attention style, fp32r matmuls = full PE speed with
~tf32 accuracy):
  phase 1: xT via PE transpose; project K^T and V(+ones col) to DRAM scratch,
           Q^T (own queries only) resident in SBUF.
  phase 2: stream K^T/V back in blocks of 4 k-chunks (kc-block OUTER loop, so
           each k block is read once); for each query slot still active, compute
           S^T = K Q^T in PSUM, exp on ScalarE (scale folded), mask via shipped
           per-core mask tiles on the final block, accumulate U = W @ [V|1] in
           PSUM, flush-add into an SBUF accumulator.
  phase 3: out = U[:, :768] * (1 / U[:, 768]) per row, DMA out.
"""

import numpy as np

import concourse.bacc as bacc
import concourse.mybir as mybir
import concourse.tile as tile
from concourse.bass_utils import run_bass_kernel_spmd
from concourse.masks import make_identity

B, S, D = 4, 4096, 768
P = 128
DC = D // P            # 6 feature chunks
NKC = S // P           # 32 key chunks
QB = 256               # query block width
NSLOT = 8              # query blocks per core (8 * 256 = 2048 queries)
SQ = NSLOT * QB        # 2048
KB = 4                 # key chunks per streamed block
NKB = NKC // KB        # 8
VW = 772               # V row width: 768 + ones col + pad
SM_SCALE = float(1.0 / np.sqrt(D))
N_CORES = 8

# Snake pairing of the 16 global query blocks between the two cores of a batch:
# slot j holds global block 2j or 2j+1; both cores run ceil-padded trip counts.
ASSIGN = [
    [(2 * j + 1) if j % 2 == 0 else (2 * j) for j in range(NSLOT)],  # role 0
    [(2 * j) if j % 2 == 0 else (2 * j + 1) for j in range(NSLOT)],  # role 1
]

ACT_F32R = True  # ScalarE may write float32r (rounding) — flip if compile rejects

_CACHE = {}


def _round_copy(nc, dst, src):
    if ACT_F32R:
        nc.scalar.copy(dst, src)
    else:
        nc.vector.tensor_copy(dst, src)


def _build_module():
    dt = mybir.dt
    f32, f32r = dt.float32, dt.float32r
    nc = bacc.Bacc("TRN2", target_bir_lowering=False, debug=False,
                   num_devices=N_CORES)

    x_d = nc.dram_tensor("x", [S, D], f32, kind="ExternalInput").ap()
    xq_d = nc.dram_tensor("xq", [SQ, D], f32, kind="ExternalInput").ap()
    wq_d = nc.dram_tensor("Wq", [D, D], f32, kind="ExternalInput").ap()
    wk_d = nc.dram_tensor("Wk", [D, D], f32, kind="ExternalInput").ap()
    wv_d = nc.dram_tensor("Wv", [D, D], f32, kind="ExternalInput").ap()
    mask_d = nc.dram_tensor("masks", [NSLOT, KB, P, QB], f32r,
                            kind="ExternalInput").ap()
    out_d = nc.dram_tensor("out", [SQ, D], f32, kind="ExternalOutput").ap()

    x_r = x_d.rearrange("(a p) d -> p a d", p=P)      # [128, 32, 768]
    xq_r = xq_d.rearrange("(a p) d -> p a d", p=P)    # [128, 16, 768]
    out_r = out_d.rearrange("(a p) d -> p a d", p=P)  # [128, 16, 768]

    Exp = mybir.ActivationFunctionType.Exp

    with tile.TileContext(nc) as tc:
        with tc.tile_pool(name="singles", bufs=1) as singles, \
             tc.tile_pool(name="dram", bufs=1, space="DRAM") as dram:
            ident = singles.tile([P, P], f32)
            make_identity(nc, ident)
            qt = singles.tile([P, DC, SQ], f32r)      # Q^T resident
            kt_t = dram.tile([P, DC, S], f32r)        # K^T scratch
            v_t = dram.tile([P, NKC, VW], f32r)       # V(+ones) scratch

            # ---------------- phase 1: projections ----------------
            with tc.tile_pool(name="wload", bufs=1) as wload, \
                 tc.tile_pool(name="weights", bufs=1) as weights, \
                 tc.tile_pool(name="p1", bufs=2) as p1, \
                 tc.tile_pool(name="stage", bufs=3) as stage, \
                 tc.tile_pool(name="pst", bufs=2, space="PSUM") as pst, \
                 tc.tile_pool(name="psp", bufs=2, space="PSUM") as psp, \
                 tc.tile_pool(name="psv", bufs=2, space="PSUM") as psv:
                w_r = {}
                for name, wd in (("wq", wq_d), ("wk", wk_d), ("wv", wv_d)):
                    wf = wload.tile([P, DC, D], f32, tag="wf")
                    nc.sync.dma_start(out=wf, in_=wd.rearrange("(c p) e -> p c e", p=P))
                    wr = weights.tile([P, DC, D], f32r, tag="w_" + name)
                    nc.vector.tensor_copy(wr, wf)
                    w_r[name] = wr

                def proj_block(src_r, a0, is_q):
                    # one 512-row block of x (or xq): transpose + project
                    xs = p1.tile([P, 4, D], f32, tag="xs")
                    nc.sync.dma_start(out=xs, in_=src_r[:, a0:a0 + 4, :])
                    xt = p1.tile([P, DC, 4 * P], f32r, tag="xt")
                    for sc in range(4):
                        for c in range(DC):
                            pt = pst.tile([P, P], f32, tag="pt")
                            nc.tensor.transpose(pt, xs[:, sc, c * P:(c + 1) * P], ident)
                            nc.vector.tensor_copy(xt[:, c, sc * P:(sc + 1) * P], pt)
                    if is_q:
                        w = w_r["wq"]
                        for ec in range(DC):
                            pp = psp.tile([P, 4 * P], f32, tag="pp")
                            for c in range(DC):
                                nc.tensor.matmul(pp, w[:, c, ec * P:(ec + 1) * P],
                                                 xt[:, c, :],
                                                 start=(c == 0), stop=(c == DC - 1))
                            _round_copy(nc, qt[:, ec, a0 * P:a0 * P + 4 * P], pp)
                    else:
                        w = w_r["wk"]
                        for ec in range(DC):
                            pp = psp.tile([P, 4 * P], f32, tag="pp")
                            for c in range(DC):
                                nc.tensor.matmul(pp, w[:, c, ec * P:(ec + 1) * P],
                                                 xt[:, c, :],
                                                 start=(c == 0), stop=(c == DC - 1))
                            stk = stage.tile([P, 4 * P], f32r, tag="stk")
                            _round_copy(nc, stk, pp)
                            nc.sync.dma_start(out=kt_t[:, ec, a0 * P:a0 * P + 4 * P],
                                              in_=stk)
                        wv = w_r["wv"]
                        for sc in range(4):
                            pv = psv.tile([P, VW], f32, tag="pv")
                            for c in range(DC):
                                lhs = xt[:, c, sc * P:(sc + 1) * P]
                                nc.tensor.matmul(pv[:, 0:512], lhs, wv[:, c, 0:512],
                                                 start=(c == 0), stop=(c == DC - 1))
                                nc.tensor.matmul(pv[:, 512:768], lhs, wv[:, c, 512:768],
                                                 start=(c == 0), stop=(c == DC - 1))
                            sv = stage.tile([P, VW], f32r, tag="sv")
                            _round_copy(nc, sv[:, 0:768], pv[:, 0:768])
                            nc.vector.memset(sv[:, 768:769].bitcast(mybir.dt.float32), 1.0)
                            nc.vector.memset(sv[:, 769:VW].bitcast(mybir.dt.float32), 0.0)
                            nc.sync.dma_start(out=v_t[:, a0 + sc, :], in_=sv)

                for blk in range(SQ // (4 * P)):
                    proj_block(xq_r, blk * 4, True)
                for blk in range(S // (4 * P)):
                    proj_block(x_r, blk * 4, False)

            # ---------------- phase 2: attention ----------------
            with tc.tile_pool(name="uaccp", bufs=1) as uaccp, \
                 tc.tile_pool(name="ring", bufs=2) as ring, \
                 tc.tile_pool(name="wtp", bufs=3) as wtp, \
                 tc.tile_pool(name="mring", bufs=2) as mring, \
                 tc.tile_pool(name="fin", bufs=2) as fin, \
                 tc.tile_pool(name="pss", bufs=3, space="PSUM") as pss, \
                 tc.tile_pool(name="psu", bufs=1, space="PSUM") as psu:
                uacc = uaccp.tile([P, 2 * NSLOT, VW], f32)
                for kb in range(NKB):
                    ktb = ring.tile([P, DC, KB * P], f32r, tag="ktb")
                    nc.sync.dma_start(out=ktb,
                                      in_=kt_t[:, :, kb * KB * P:(kb + 1) * KB * P])
                    vb = ring.tile([P, KB, VW], f32r, tag="vb")
                    nc.sync.dma_start(out=vb, in_=v_t[:, kb * KB:(kb + 1) * KB, :])
                    for j in range(kb, NSLOT):
                        mb = None
                        if kb == j:
                            mb = mring.tile([P, KB, QB], f32r, tag="mb")
                            nc.sync.dma_start(
                                out=mb, in_=mask_d[j].rearrange("t p f -> p t f"))
                        pu0 = psu.tile([P, VW], f32, tag="pu0")
                        pu1 = psu.tile([P, VW], f32, tag="pu1")
                        pus = (pu0, pu1)
                        for t in range(KB):
                            ps = pss.tile([P, QB], f32, tag="ps")
                            for c in range(DC):
                                nc.tensor.matmul(ps, ktb[:, c, t * P:(t + 1) * P],
                                                 qt[:, c, j * QB:(j + 1) * QB],
                                                 start=(c == 0), stop=(c == DC - 1))
                            wt = wtp.tile([P, QB], f32r, tag="wt")
                            if ACT_F32R:
                                nc.scalar.activation(wt, ps, Exp, scale=SM_SCALE)
                            else:
                                wtf = wtp.tile([P, QB], f32, tag="wtf")
                                nc.scalar.activation(wtf, ps, Exp, scale=SM_SCALE)
                                nc.vector.tensor_copy(wt, wtf)
                            if mb is not None:
                                nc.vector.tensor_mul(wt, wt, mb[:, t, :])
                            for qc in range(2):
                                lhs = wt[:, qc * P:(qc + 1) * P]
                                nc.tensor.matmul(pus[qc][:, 0:512], lhs,
                                                 vb[:, t, 0:512],
                                                 start=(t == 0), stop=(t == KB - 1))
                                nc.tensor.matmul(pus[qc][:, 512:VW], lhs,
                                                 vb[:, t, 512:VW],
                                                 start=(t == 0), stop=(t == KB - 1))
                        for qc in range(2):
                            dst = uacc[:, 2 * j + qc, 0:769]
                            if kb == 0:
                                nc.scalar.copy(dst, pus[qc][:, 0:769])
                            else:
                                nc.vector.tensor_add(dst, dst, pus[qc][:, 0:769])

                # ---------------- phase 3: normalize + store ----------------
                for j in range(NSLOT):
                    for qc in range(2):
                        sl = 2 * j + qc
                        zr = fin.tile([P, 1], f32, tag="zr")
                        nc.vector.reciprocal(zr, uacc[:, sl, 768:769])
                        ob = fin.tile([P, D], f32, tag="ob")
                        nc.scalar.mul(ob, uacc[:, sl, 0:768], zr)
                        nc.sync.dma_start(out=out_r[:, sl, :], in_=ob)

    nc.compile()
    return nc


def _get_module():
    if "nc" not in _CACHE:
        _CACHE["nc"] = _build_module()
    return _CACHE["nc"]


def _build_masks(chunks):
    m = np.zeros((NSLOT, KB, P, QB), np.float32)
    prow = np.arange(P)[:, None]
    fcol = np.arange(QB)[None, :]
    for j, g in enumerate(chunks):
        for t in range(KB):
            kc = KB * j + t
            m[j, t] = (prow <= fcol + (g * QB - kc * P)).astype(np.float32)
    return m


def _make_in_maps(inputs):
    x = np.asarray(inputs["x"], np.float32)
    Wq = np.ascontiguousarray(np.asarray(inputs["Wq"], np.float32))
    Wk = np.ascontiguousarray(np.asarray(inputs["Wk"], np.float32))
    Wv = np.ascontiguousarray(np.asarray(inputs["Wv"], np.float32))
    in_maps = []
    for c in range(N_CORES):
        b, r = c // 2, c % 2
        chunks = ASSIGN[r]
        xb = np.ascontiguousarray(x[b])
        xq = np.ascontiguousarray(
            xb.reshape(S // QB, QB, D)[chunks].reshape(SQ, D))
        in_maps.append({
            "x": xb, "xq": xq, "Wq": Wq, "Wk": Wk, "Wv": Wv,
            "masks": _build_masks(chunks),
        })
    return in_maps


def _run(inputs, trace=False, trace_kwargs=None):
    nc = _get_module()
    in_maps = _make_in_maps(inputs)

    kw = {}
    if trace:
        kw["trace"] = True
        kw["trace_cores"] = (trace_kwargs or {}).pop("trace_cores", None) \
            or list(range(N_CORES))
        if trace_kwargs:
            kw["trace_kwargs"] = trace_kwargs
    res = run_bass_kernel_spmd(nc, in_maps, core_ids=list(range(N_CORES)), **kw)

    out = np.empty((B, S, D), np.float32)
    for c in range(N_CORES):
        b, r = c // 2, c % 2
        o = res.results[c]["out"].reshape(NSLOT, QB, D)
        for j, g in enumerate(ASSIGN[r]):
            out[b, g * QB:(g + 1) * QB] = o[j]
    return out, res


def kernel(**inputs) -> np.ndarray:
    out, _ = _run(inputs, trace=False)
    return out
